# revision 11
# baseline (speedup 1.0000x reference)
"""DigitCaps dynamic-routing kernel for 8 Trainium2 NeuronCores.

Sharding: J (num_capsule=32) split 8 ways -> 4 capsules per core, batch
replicated. W is SBUF-resident in two layouts: wn ([iw, (ich,q,j,p)]) for
the i-contraction GEMMs, wt ([(j,p), (g,q,iw)]) as the stationary operand
of the routing T-matmuls. The routing softmax over J uses a cross-core
AllReduce of per-(b,i) partial exp sums; a renormalization-invariance
trick keeps a single running tensor F (= c, up to a shared normalizer).

b-pass structure (per g-block of 128 i's, per pair of 2 capsules):
  T'[iw, (q, jj, b)] = matmul(lhsT=wt[:, g, q, :], rhs=vbd_pair)  (PE)
  u = T' * x  (ACT evacuates PSUM->SBUF bf16, DVE multiplies at 2x,
               x comes straight from the resident xq layout)
  Delta[iw, (jj, b)] = tree-sum_q u   (DVE, lands in F's layout: no
                                       transposes anywhere)
  F *= exp(Delta)  (ACT exp)
"""

import numpy as np
import ml_dtypes

import concourse.bacc as bacc
import concourse.mybir as mybir
import concourse.tile as tile
from concourse.bass_utils import run_bass_kernel_spmd
from concourse.masks import make_identity

BF16 = mybir.dt.bfloat16
F32 = mybir.dt.float32
NP_BF16 = ml_dtypes.bfloat16

N_CORES = 8
B = 64
I = 2048
Q = 16
J = 32
P = 32
JL = J // N_CORES
ICH = I // 128
EPS = 1e-7
AF = mybir.ActivationFunctionType

_CACHED = {}


def _squash(nc, small, v_sb, eps_ap):
    """In-place squash over p of v_sb [64, JL*P] fp32 (free = (j, p))."""
    sq = small.tile([B, JL * P], F32, tag="sq")
    nc.vector.tensor_mul(sq[:], v_sb[:], v_sb[:])
    red = sq.rearrange("b (j p) -> b j p", j=JL)
    w = P
    while w > 1:
        h = w // 2
        nc.vector.tensor_add(red[:, :, 0:h], red[:, :, 0:h], red[:, :, h:w])
        w = h
    s2 = small.tile([B, JL], F32, tag="s2")
    nc.vector.tensor_copy(s2[:], red[:, :, 0])
    # sqrt via exp(0.5*ln): stays in the natural_log_exp ACT table set,
    # avoiding a ~2.7us table reload around every routing exp
    lg = small.tile([B, JL], F32, tag="lg")
    nc.scalar.activation(lg[:], s2[:], AF.Ln, bias=eps_ap[:B, :])
    rt = small.tile([B, JL], F32, tag="rt")
    nc.scalar.activation(rt[:], lg[:], AF.Exp, scale=0.5)
    den = small.tile([B, JL], F32, tag="den")
    nc.vector.tensor_mul(den[:], s2[:], rt[:])
    nc.vector.tensor_add(den[:], den[:], rt[:])
    rec = small.tile([B, JL], F32, tag="rec")
    nc.vector.reciprocal(rec[:], den[:])
    scale = small.tile([B, JL], F32, tag="scale")
    nc.vector.tensor_mul(scale[:], s2[:], rec[:])
    vv = v_sb.rearrange("b (j p) -> b j p", j=JL)
    sc_b = scale.unsqueeze(2).broadcast_to([B, JL, P])
    nc.vector.tensor_mul(vv[:], vv[:], sc_b[:])


def _build_vbd(nc, small, psum_t, v_sb, identity):
    """v_sb [64, (j,p)] fp32 -> two block-diag bf16 rhs [128, (jj 2, b 64)]."""
    vt_ps = psum_t.tile([128, B], F32, tag="vt_ps")
    nc.tensor.transpose(vt_ps[:], v_sb[:], identity[:B, :B])
    vt = small.tile([128, B], F32, tag="vt")
    nc.scalar.copy(vt[:], vt_ps[:])  # [(j,p), b]
    vbds = []
    for pair in range(2):
        vbd = small.tile([128, 2 * B], BF16, tag=f"vbd{pair}")
        nc.vector.memset(vbd[:], 0.0)
        for jj in range(2):
            j = pair * 2 + jj
            nc.vector.tensor_copy(
                vbd[j * P:(j + 1) * P, jj * B:(jj + 1) * B],
                vt[j * P:(j + 1) * P, :],
            )
        vbds.append(vbd)
    return vbds


def _vT_to_v(nc, small, ps_vt, vT_ps, identity, scale=None):
    """vT psum [128 (j,p), 64 b] -> v_sb [64, (j,p)] fp32 via evac+transpose."""
    vT = small.tile([128, B], F32, tag="vTe")
    if scale is None:
        nc.scalar.copy(vT[:], vT_ps[:])
    else:
        nc.scalar.mul(vT[:], vT_ps[:], scale)
    v_ps = ps_vt.tile([B, 128], F32, tag="v_ps2")
    nc.tensor.transpose(v_ps[:], vT[:], identity[:])
    v_sb = small.tile([B, JL * P], F32, tag="v")
    nc.scalar.copy(v_sb[:], v_ps[:])
    return v_sb


def build_kernel():
    if "nc" in _CACHED:
        return _CACHED["nc"]
    nc = bacc.Bacc(
        "TRN2", target_bir_lowering=False, debug=False, num_devices=N_CORES
    )
    wn_d = nc.dram_tensor("wn", [128, ICH * Q * JL * P], BF16, kind="ExternalInput")
    wt_d = nc.dram_tensor("wt", [128, I * Q], BF16, kind="ExternalInput")
    xq_d = nc.dram_tensor("xq", [128, ICH * Q * B], BF16, kind="ExternalInput")
    out_d = nc.dram_tensor("o", [B, JL * P], F32, kind="ExternalOutput")

    with tile.TileContext(nc) as tc:
        with (
            tc.tile_pool(name="big", bufs=1) as big,
            tc.tile_pool(name="ustr", bufs=4) as ustr,
            tc.tile_pool(name="small", bufs=1) as small,
            tc.tile_pool(name="ytile", bufs=4) as ytile,
            tc.tile_pool(name="dram", bufs=4, space="DRAM") as dram,
        ):
            # ---- resident loads -------------------------------------
            # xq + wn first (S-pass needs only these); wt can land later,
            # under the S-pass / first b-pass
            xq = big.tile([128, ICH * Q * B], BF16, tag="xq")        # 32K/part
            nc.sync.dma_start(xq[:], xq_d[:])
            xqv = xq.rearrange("k (ich q b) -> k ich q b", ich=ICH, q=Q)
            wn = big.tile([128, ICH * Q * JL * P], BF16, tag="wn")   # 64K/part
            nc.sync.dma_start(wn[:], wn_d[:])
            wnv = wn.rearrange("k (ich q j p) -> k ich q j p", ich=ICH, q=Q, j=JL)
            wt = big.tile([128, I * Q], BF16, tag="wt")              # 64K/part
            nc.sync.dma_start(wt[:], wt_d[:])
            wtv = wt.rearrange("k (g q i) -> k g q i", g=ICH, q=Q)

            identity = big.tile([128, 128], F32, tag="ident")
            make_identity(nc, identity[:])
            eps_t = big.tile([128, 1], F32, tag="eps")
            nc.vector.memset(eps_t[:], EPS)

            # F[iw, (ich, j, b)] bf16: running c (up to global normalizer)
            f_sb = big.tile([128, ICH * JL * B], BF16, tag="f")      # 8K/part
            f_v = f_sb.rearrange("k (ich j b) -> k ich j b", ich=ICH, j=JL)

            # warmup collective to absorb core-start skew
            wu_s = small.tile([128, 8], F32, tag="wu")
            nc.gpsimd.memset(wu_s[:], 0.0)
            wu_i = dram.tile([128, 8], F32, tag="wu_i")
            wu_o = dram.tile([128, 8], F32, tag="wu_o")
            nc.gpsimd.dma_start(wu_i[:], wu_s[:])
            nc.gpsimd.collective_compute(
                "AllReduce", mybir.AluOpType.add,
                replica_groups=[list(range(N_CORES))],
                ins=[wu_i.opt()], outs=[wu_o.opt()],
            )

            # ---- S-pass: vT[(j,p), b] = sum_{i,q} W x ---------------
            with tc.tile_pool(name="ps_s", bufs=1, space="PSUM") as ps_s, \
                 tc.tile_pool(name="ps_st", bufs=1, space="PSUM") as ps_st:
                s_ps = ps_s.tile([128, B], F32, tag="s_ps")
                n_mm = ICH * Q
                k = 0
                for ich in range(ICH):
                    for q in range(Q):
                        nc.tensor.matmul(
                            s_ps[:],
                            wnv[:, ich, q, :, :],       # lhsT [128, (j p)]
                            xqv[:, ich, q, :],          # rhs  [128, 64]
                            start=(k == 0), stop=(k == n_mm - 1),
                        )
                        k += 1
                v_sb = _vT_to_v(nc, small, ps_st, s_ps, identity, scale=1.0 / J)
                _squash(nc, small, v_sb, eps_t)
                vbds = _build_vbd(nc, small, ps_st, v_sb, identity)

            # ---- 2 routing iterations -------------------------------
            for it in range(2):
                first = it == 0
                # b-pass: T'[iw, (q, jj, b)] per (g, pair), wt stationary
                cc_pend = [None, None]
                with tc.tile_pool(name=f"ps_b{it}", bufs=2, space="PSUM") as ps_b:
                    for g in range(ICH):
                        t_pss = [
                            ps_b.tile(
                                [128, Q * 2 * B], F32, tag="t_ps",
                                name=f"t_ps{it}_{g}_{pair}",
                            )
                            for pair in range(2)
                        ]
                        # interleave pairs so each wt slice is loaded once
                        for q in range(Q):
                            for pair in range(2):
                                nc.tensor.matmul(
                                    t_pss[pair][:, q * 128:(q + 1) * 128],
                                    wtv[:, g, q, :],        # lhsT [128, 128]
                                    vbds[pair][:],          # rhs  [128, 128]
                                    start=True, stop=True,
                                )
                        for pair in range(2):
                            t_ps = t_pss[pair]
                            u = ustr.tile(
                                [128, Q * 2 * B], BF16, tag="u",
                                name=f"u{it}_{g}_{pair}",
                            )
                            # ACT evacuates PSUM (frees banks), DVE muls 2x
                            nc.scalar.copy(u[:], t_ps[:])
                            uv = u.rearrange("k (q j b) -> k q j b", q=Q, j=2)
                            xb = (
                                xqv[:, g, :, :]
                                .unsqueeze(2).broadcast_to([128, Q, 2, B])
                            )
                            nc.vector.tensor_mul(uv[:], uv[:], xb[:])
                            # tree-reduce over q -> Delta [iw, (jj, b)]
                            w = Q
                            while w > 1:
                                h = w // 2
                                nc.vector.tensor_add(
                                    u[:, 0:h * 128], u[:, 0:h * 128],
                                    u[:, h * 128:w * 128],
                                )
                                w = h
                            off = (g * JL + pair * 2) * B
                            dst = f_sb[:, off:off + 2 * B]
                            if first:
                                nc.scalar.activation(dst, u[:, 0:128], AF.Exp)
                            else:
                                ex = ustr.tile([128, 128], BF16, tag="ex")
                                nc.scalar.activation(ex[:], u[:, 0:128], AF.Exp)
                                nc.gpsimd.tensor_mul(dst, dst, ex[:])
                        if g == 7 or g == ICH - 1:
                            h = 0 if g == 7 else 1
                            sl = slice(h * 8, h * 8 + 8)
                            zph = small.tile(
                                [128, 8 * B], F32, tag=f"zp{h}",
                                name=f"zp{it}_{h}",
                            )
                            zpv = zph.rearrange("k (ic b) -> k ic b", ic=8)
                            nc.gpsimd.tensor_add(
                                zpv[:], f_v[:, sl, 0, :], f_v[:, sl, 1, :]
                            )
                            for j in range(2, JL):
                                nc.gpsimd.tensor_add(
                                    zpv[:], zpv[:], f_v[:, sl, j, :]
                                )
                            cc_i = dram.tile(
                                [128, 8 * B], F32, tag=f"cc_i{h}",
                                name=f"cci{it}_{h}",
                            )
                            cc_o = dram.tile(
                                [128, 8 * B], F32, tag=f"cc_o{h}",
                                name=f"cco{it}_{h}",
                            )
                            nc.gpsimd.dma_start(cc_i[:], zph[:])
                            nc.gpsimd.collective_compute(
                                "AllReduce", mybir.AluOpType.add,
                                replica_groups=[list(range(N_CORES))],
                                ins=[cc_i.opt()], outs=[cc_o.opt()],
                            )
                            cc_pend[h] = cc_o

                # v-pass: vT[(j,p), b] = sum_{i,q} W (F x), col-tiled over j
                with tc.tile_pool(name=f"ps_v{it}", bufs=1, space="PSUM") as ps_v, \
                     tc.tile_pool(name=f"ps_vt{it}", bufs=2, space="PSUM") as ps_vt:
                    vT_ps = ps_v.tile([128, B], F32, tag="vT_ps")
                    # process i-halves as their AllReduce results land, so
                    # half-1's collective overlaps half-0's y-mul + matmuls
                    for h in range(2):
                        sl = slice(h * 8, h * 8 + 8)
                        zh = small.tile(
                            [128, 8 * B], F32, tag=f"z{h}", name=f"z{it}_{h}"
                        )
                        nc.sync.dma_start(zh[:], cc_pend[h][:])
                        nc.vector.reciprocal(zh[:], zh[:])
                        zb = small.tile(
                            [128, 8 * B], BF16, tag=f"zb{h}",
                            name=f"zb{it}_{h}",
                        )
                        nc.vector.tensor_copy(zb[:], zh[:])
                        zrv = zb.rearrange("k (ic b) -> k ic b", ic=8)
                        for j in range(JL):
                            nc.gpsimd.tensor_mul(
                                f_v[:, sl, j, :], f_v[:, sl, j, :], zrv[:]
                            )
                        for ich in range(h * 8, h * 8 + 8):
                            ys = []
                            for j in range(JL):
                                y = ytile.tile(
                                    [128, Q * B], BF16, tag="y",
                                    name=f"y{it}_{ich}_{j}",
                                )
                                yv = y.rearrange("k (q b) -> k q b", q=Q)
                                cb = (
                                    f_v[:, ich, j, :]
                                    .unsqueeze(1).broadcast_to([128, Q, B])
                                )
                                nc.vector.tensor_mul(
                                    yv[:], xqv[:, ich, :, :], cb[:]
                                )
                                ys.append(y)
                            for q in range(Q):
                                for j in range(JL):
                                    nc.tensor.matmul(
                                        vT_ps[j * P:(j + 1) * P, :],
                                        wnv[:, ich, q, j, :],
                                        ys[j][:, q * B:(q + 1) * B],
                                        start=(ich == 0 and q == 0),
                                        stop=(ich == ICH - 1 and q == Q - 1),
                                        tile_position=(0, j * P),
                                    )
                    v_sb = _vT_to_v(nc, small, ps_vt, vT_ps, identity)
                    _squash(nc, small, v_sb, eps_t)
                    if it == 0:
                        vbds = _build_vbd(nc, small, ps_vt, v_sb, identity)
                    else:
                        nc.sync.dma_start(out_d[:], v_sb[:])

    nc.compile()
    _CACHED["nc"] = nc
    return nc


def _prep_inputs(inputs_np, W_np):
    x = np.ascontiguousarray(inputs_np)           # [B, I, Q] f32
    W = np.ascontiguousarray(W_np)                # [J, I, P, Q] f32
    xq = (
        x.reshape(B, ICH, 128, Q).transpose(2, 1, 3, 0)
        .astype(NP_BF16).reshape(128, ICH * Q * B)
    )
    in_maps = []
    for r in range(N_CORES):
        Wr = W[r * JL:(r + 1) * JL]                       # [4, I, P, Q]
        wn = (
            Wr.reshape(JL, ICH, 128, P, Q).transpose(2, 1, 4, 0, 3)
            .astype(NP_BF16).reshape(128, ICH * Q * JL * P)
        )
        wt = (
            Wr.reshape(JL, ICH, 128, P, Q)
            .transpose(0, 3, 1, 4, 2)                     # [j, p, g, q, iw]
            .astype(NP_BF16).reshape(128, I * Q)
        )
        in_maps.append(
            {
                "wn": np.ascontiguousarray(wn),
                "wt": np.ascontiguousarray(wt),
                "xq": np.ascontiguousarray(xq),
            }
        )
    return in_maps


def kernel(inputs, W, _trace=False):
    nc = build_kernel()
    in_maps = _prep_inputs(np.asarray(inputs), np.asarray(W))
    res = run_bass_kernel_spmd(nc, in_maps, list(range(N_CORES)), trace=_trace)
    out = np.concatenate(
        [res.results[r]["o"].reshape(B, JL, P) for r in range(N_CORES)], axis=1
    )
    if _trace:
        kernel.last_exec_ns = res.exec_time_ns
        kernel.last_results = res
    return out.astype(np.float32)


# revision 12
# speedup vs baseline: 1.1623x; 1.1623x over previous
"""DigitCaps dynamic-routing kernel for 8 Trainium2 NeuronCores.

Sharding: J (num_capsule=32) split 8 ways -> 4 capsules per core, batch
replicated. W is SBUF-resident in two layouts: wn ([iw, (ich,q,j,p)]) for
the i-contraction GEMMs, wt ([(j,p), (g,q,iw)]) as the stationary operand
of the routing T-matmuls. The routing softmax over J uses a cross-core
AllReduce of per-(b,i) partial exp sums; a renormalization-invariance
trick keeps a single running tensor F (= c, up to a shared normalizer).

b-pass structure (per g-block of 128 i's, per pair of 2 capsules):
  T'[iw, (q, jj, b)] = matmul(lhsT=wt[:, g, q, :], rhs=vbd_pair)  (PE)
  u = T' * x  (ACT evacuates PSUM->SBUF bf16, DVE multiplies at 2x,
               x comes straight from the resident xq layout)
  Delta[iw, (jj, b)] = tree-sum_q u   (DVE, lands in F's layout: no
                                       transposes anywhere)
  F *= exp(Delta)  (ACT exp)
"""

import numpy as np
import ml_dtypes

import concourse.bacc as bacc
import concourse.mybir as mybir
import concourse.tile as tile
from concourse.bass_utils import run_bass_kernel_spmd
from concourse.masks import make_identity

BF16 = mybir.dt.bfloat16
F32 = mybir.dt.float32
NP_BF16 = ml_dtypes.bfloat16

N_CORES = 8
B = 64
I = 2048
Q = 16
J = 32
P = 32
JL = J // N_CORES
ICH = I // 128
EPS = 1e-7
AF = mybir.ActivationFunctionType

_CACHED = {}


def _squash(nc, small, v_sb, eps_ap):
    """In-place squash over p of v_sb [64, JL*P] fp32 (free = (j, p))."""
    sq = small.tile([B, JL * P], F32, tag="sq")
    nc.vector.tensor_mul(sq[:], v_sb[:], v_sb[:])
    red = sq.rearrange("b (j p) -> b j p", j=JL)
    w = P
    while w > 1:
        h = w // 2
        nc.vector.tensor_add(red[:, :, 0:h], red[:, :, 0:h], red[:, :, h:w])
        w = h
    s2 = small.tile([B, JL], F32, tag="s2")
    nc.vector.tensor_copy(s2[:], red[:, :, 0])
    # sqrt via exp(0.5*ln): stays in the natural_log_exp ACT table set,
    # avoiding a ~2.7us table reload around every routing exp
    lg = small.tile([B, JL], F32, tag="lg")
    nc.scalar.activation(lg[:], s2[:], AF.Ln, bias=eps_ap[:B, :])
    rt = small.tile([B, JL], F32, tag="rt")
    nc.scalar.activation(rt[:], lg[:], AF.Exp, scale=0.5)
    den = small.tile([B, JL], F32, tag="den")
    nc.vector.tensor_mul(den[:], s2[:], rt[:])
    nc.vector.tensor_add(den[:], den[:], rt[:])
    rec = small.tile([B, JL], F32, tag="rec")
    nc.vector.reciprocal(rec[:], den[:])
    scale = small.tile([B, JL], F32, tag="scale")
    nc.vector.tensor_mul(scale[:], s2[:], rec[:])
    vv = v_sb.rearrange("b (j p) -> b j p", j=JL)
    sc_b = scale.unsqueeze(2).broadcast_to([B, JL, P])
    nc.vector.tensor_mul(vv[:], vv[:], sc_b[:])


def _build_vbd(nc, small, psum_t, v_sb, identity):
    """v_sb [64, (j,p)] fp32 -> two block-diag bf16 rhs [128, (jj 2, b 64)]."""
    vt_ps = psum_t.tile([128, B], F32, tag="vt_ps")
    nc.tensor.transpose(vt_ps[:], v_sb[:], identity[:B, :B])
    vt = small.tile([128, B], F32, tag="vt")
    nc.scalar.copy(vt[:], vt_ps[:])  # [(j,p), b]
    vbds = []
    for pair in range(2):
        vbd = small.tile([128, 2 * B], BF16, tag=f"vbd{pair}")
        nc.vector.memset(vbd[:], 0.0)
        for jj in range(2):
            j = pair * 2 + jj
            nc.vector.tensor_copy(
                vbd[j * P:(j + 1) * P, jj * B:(jj + 1) * B],
                vt[j * P:(j + 1) * P, :],
            )
        vbds.append(vbd)
    return vbds


def _vT_to_v(nc, small, ps_vt, vT_ps, identity, scale=None):
    """vT psum [128 (j,p), 64 b] -> v_sb [64, (j,p)] fp32 via evac+transpose."""
    vT = small.tile([128, B], F32, tag="vTe")
    if scale is None:
        nc.scalar.copy(vT[:], vT_ps[:])
    else:
        nc.scalar.mul(vT[:], vT_ps[:], scale)
    v_ps = ps_vt.tile([B, 128], F32, tag="v_ps2")
    nc.tensor.transpose(v_ps[:], vT[:], identity[:])
    v_sb = small.tile([B, JL * P], F32, tag="v")
    nc.scalar.copy(v_sb[:], v_ps[:])
    return v_sb


def build_kernel():
    if "nc" in _CACHED:
        return _CACHED["nc"]
    nc = bacc.Bacc(
        "TRN2", target_bir_lowering=False, debug=False, num_devices=N_CORES
    )
    wn_d = nc.dram_tensor("wn", [128, ICH * Q * JL * P], BF16, kind="ExternalInput")
    wt_d = nc.dram_tensor("wt", [128, I * Q], BF16, kind="ExternalInput")
    xq_d = nc.dram_tensor("xq", [128, ICH * Q * B], BF16, kind="ExternalInput")
    out_d = nc.dram_tensor("o", [B, JL * P], F32, kind="ExternalOutput")

    with tile.TileContext(nc) as tc:
        with (
            tc.tile_pool(name="big", bufs=1) as big,
            tc.tile_pool(name="ustr", bufs=4) as ustr,
            tc.tile_pool(name="small", bufs=1) as small,
            tc.tile_pool(name="ytile", bufs=4) as ytile,
            tc.tile_pool(name="dram", bufs=4, space="DRAM") as dram,
        ):
            # ---- resident loads -------------------------------------
            # xq + wn first (S-pass needs only these); wt can land later,
            # under the S-pass / first b-pass
            xq = big.tile([128, ICH * Q * B], BF16, tag="xq")        # 32K/part
            nc.sync.dma_start(xq[:], xq_d[:])
            xqv = xq.rearrange("k (ich q b) -> k ich q b", ich=ICH, q=Q)
            wn = big.tile([128, ICH * Q * JL * P], BF16, tag="wn")   # 64K/part
            nc.sync.dma_start(wn[:], wn_d[:])
            wnv = wn.rearrange("k (ich q j p) -> k ich q j p", ich=ICH, q=Q, j=JL)
            wt = big.tile([128, I * Q], BF16, tag="wt")              # 64K/part
            nc.sync.dma_start(wt[:], wt_d[:])
            wtv = wt.rearrange("k (g q i) -> k g q i", g=ICH, q=Q)

            identity = big.tile([128, 128], F32, tag="ident")
            make_identity(nc, identity[:])
            eps_t = big.tile([128, 1], F32, tag="eps")
            nc.vector.memset(eps_t[:], EPS)

            # F[iw, (ich, j, b)] bf16: running c (up to global normalizer)
            f_sb = big.tile([128, ICH * JL * B], BF16, tag="f")      # 8K/part
            f_v = f_sb.rearrange("k (ich j b) -> k ich j b", ich=ICH, j=JL)

            # warmup collective to absorb core-start skew
            wu_s = small.tile([128, 8], F32, tag="wu")
            nc.gpsimd.memset(wu_s[:], 0.0)
            wu_i = dram.tile([128, 8], F32, tag="wu_i")
            wu_o = dram.tile([128, 8], F32, tag="wu_o")
            nc.gpsimd.dma_start(wu_i[:], wu_s[:])
            nc.gpsimd.collective_compute(
                "AllReduce", mybir.AluOpType.add,
                replica_groups=[list(range(N_CORES))],
                ins=[wu_i.opt()], outs=[wu_o.opt()],
            )

            # ---- S-pass: vT[(j,p), b] = sum_{i,q} W x ---------------
            with tc.tile_pool(name="ps_s", bufs=1, space="PSUM") as ps_s, \
                 tc.tile_pool(name="ps_st", bufs=1, space="PSUM") as ps_st:
                s_ps = ps_s.tile([128, B], F32, tag="s_ps")
                n_mm = ICH * Q
                k = 0
                for ich in range(ICH):
                    for q in range(Q):
                        nc.tensor.matmul(
                            s_ps[:],
                            wnv[:, ich, q, :, :],       # lhsT [128, (j p)]
                            xqv[:, ich, q, :],          # rhs  [128, 64]
                            start=(k == 0), stop=(k == n_mm - 1),
                        )
                        k += 1
                v_sb = _vT_to_v(nc, small, ps_st, s_ps, identity, scale=1.0 / J)
                _squash(nc, small, v_sb, eps_t)
                vbds = _build_vbd(nc, small, ps_st, v_sb, identity)

            # ---- 2 routing iterations -------------------------------
            for it in range(2):
                first = it == 0
                # b-pass: T'[iw, (q, jj, b)] per (g, pair), wt stationary
                cc_pend = [None, None]
                with tc.tile_pool(name=f"ps_b{it}", bufs=2, space="PSUM") as ps_b:
                    for g in range(ICH):
                        t_pss = [
                            ps_b.tile(
                                [128, Q * 2 * B], F32, tag="t_ps",
                                name=f"t_ps{it}_{g}_{pair}",
                            )
                            for pair in range(2)
                        ]
                        # interleave pairs so each wt slice is loaded once
                        for q in range(Q):
                            for pair in range(2):
                                nc.tensor.matmul(
                                    t_pss[pair][:, q * 128:(q + 1) * 128],
                                    wtv[:, g, q, :],        # lhsT [128, 128]
                                    vbds[pair][:],          # rhs  [128, 128]
                                    start=True, stop=True,
                                )
                        for pair in range(2):
                            t_ps = t_pss[pair]
                            u = ustr.tile(
                                [128, Q * 2 * B], BF16, tag="u",
                                name=f"u{it}_{g}_{pair}",
                            )
                            # ACT evacuates PSUM (frees banks), DVE muls 2x
                            nc.scalar.copy(u[:], t_ps[:])
                            uv = u.rearrange("k (q j b) -> k q j b", q=Q, j=2)
                            xb = (
                                xqv[:, g, :, :]
                                .unsqueeze(2).broadcast_to([128, Q, 2, B])
                            )
                            nc.vector.tensor_mul(uv[:], uv[:], xb[:])
                            # tree-reduce over q -> Delta [iw, (jj, b)]
                            w = Q
                            while w > 1:
                                h = w // 2
                                nc.vector.tensor_add(
                                    u[:, 0:h * 128], u[:, 0:h * 128],
                                    u[:, h * 128:w * 128],
                                )
                                w = h
                            off = (g * JL + pair * 2) * B
                            dst = f_sb[:, off:off + 2 * B]
                            if first:
                                nc.scalar.activation(dst, u[:, 0:128], AF.Exp)
                            else:
                                ex = ustr.tile([128, 128], BF16, tag="ex")
                                nc.scalar.activation(ex[:], u[:, 0:128], AF.Exp)
                                nc.vector.tensor_mul(dst, dst, ex[:])
                        if g == 7 or g == ICH - 1:
                            h = 0 if g == 7 else 1
                            sl = slice(h * 8, h * 8 + 8)
                            zph = small.tile(
                                [128, 8 * B], F32, tag=f"zp{h}",
                                name=f"zp{it}_{h}",
                            )
                            zpv = zph.rearrange("k (ic b) -> k ic b", ic=8)
                            nc.vector.tensor_add(
                                zpv[:], f_v[:, sl, 0, :], f_v[:, sl, 1, :]
                            )
                            for j in range(2, JL):
                                nc.vector.tensor_add(
                                    zpv[:], zpv[:], f_v[:, sl, j, :]
                                )
                            cc_i = dram.tile(
                                [128, 8 * B], F32, tag=f"cc_i{h}",
                                name=f"cci{it}_{h}",
                            )
                            cc_o = dram.tile(
                                [128, 8 * B], F32, tag=f"cc_o{h}",
                                name=f"cco{it}_{h}",
                            )
                            nc.gpsimd.dma_start(cc_i[:], zph[:])
                            nc.gpsimd.collective_compute(
                                "AllReduce", mybir.AluOpType.add,
                                replica_groups=[list(range(N_CORES))],
                                ins=[cc_i.opt()], outs=[cc_o.opt()],
                            )
                            cc_pend[h] = cc_o

                # v-pass: vT[(j,p), b] = sum_{i,q} W (F x), col-tiled over j
                with tc.tile_pool(name=f"ps_v{it}", bufs=1, space="PSUM") as ps_v, \
                     tc.tile_pool(name=f"ps_vt{it}", bufs=2, space="PSUM") as ps_vt:
                    vT_ps = ps_v.tile([128, B], F32, tag="vT_ps")
                    # process i-halves as their AllReduce results land, so
                    # half-1's collective overlaps half-0's y-mul + matmuls
                    for h in range(2):
                        sl = slice(h * 8, h * 8 + 8)
                        zh = small.tile(
                            [128, 8 * B], F32, tag=f"z{h}", name=f"z{it}_{h}"
                        )
                        nc.sync.dma_start(zh[:], cc_pend[h][:])
                        nc.vector.reciprocal(zh[:], zh[:])
                        zb = small.tile(
                            [128, 8 * B], BF16, tag=f"zb{h}",
                            name=f"zb{it}_{h}",
                        )
                        nc.vector.tensor_copy(zb[:], zh[:])
                        zrv = zb.rearrange("k (ic b) -> k ic b", ic=8)
                        for j in range(JL):
                            nc.vector.tensor_mul(
                                f_v[:, sl, j, :], f_v[:, sl, j, :], zrv[:]
                            )
                        for ich in range(h * 8, h * 8 + 8):
                            ys = []
                            for j in range(JL):
                                y = ytile.tile(
                                    [128, Q * B], BF16, tag="y",
                                    name=f"y{it}_{ich}_{j}",
                                )
                                yv = y.rearrange("k (q b) -> k q b", q=Q)
                                cb = (
                                    f_v[:, ich, j, :]
                                    .unsqueeze(1).broadcast_to([128, Q, B])
                                )
                                nc.vector.tensor_mul(
                                    yv[:], xqv[:, ich, :, :], cb[:]
                                )
                                ys.append(y)
                            for q in range(Q):
                                for j in range(JL):
                                    nc.tensor.matmul(
                                        vT_ps[j * P:(j + 1) * P, :],
                                        wnv[:, ich, q, j, :],
                                        ys[j][:, q * B:(q + 1) * B],
                                        start=(ich == 0 and q == 0),
                                        stop=(ich == ICH - 1 and q == Q - 1),
                                        tile_position=(0, j * P),
                                    )
                    v_sb = _vT_to_v(nc, small, ps_vt, vT_ps, identity)
                    _squash(nc, small, v_sb, eps_t)
                    if it == 0:
                        vbds = _build_vbd(nc, small, ps_vt, v_sb, identity)
                    else:
                        nc.sync.dma_start(out_d[:], v_sb[:])

    nc.compile()
    _CACHED["nc"] = nc
    return nc


def _prep_inputs(inputs_np, W_np):
    x = np.ascontiguousarray(inputs_np)           # [B, I, Q] f32
    W = np.ascontiguousarray(W_np)                # [J, I, P, Q] f32
    xq = (
        x.reshape(B, ICH, 128, Q).transpose(2, 1, 3, 0)
        .astype(NP_BF16).reshape(128, ICH * Q * B)
    )
    in_maps = []
    for r in range(N_CORES):
        Wr = W[r * JL:(r + 1) * JL]                       # [4, I, P, Q]
        wn = (
            Wr.reshape(JL, ICH, 128, P, Q).transpose(2, 1, 4, 0, 3)
            .astype(NP_BF16).reshape(128, ICH * Q * JL * P)
        )
        wt = (
            Wr.reshape(JL, ICH, 128, P, Q)
            .transpose(0, 3, 1, 4, 2)                     # [j, p, g, q, iw]
            .astype(NP_BF16).reshape(128, I * Q)
        )
        in_maps.append(
            {
                "wn": np.ascontiguousarray(wn),
                "wt": np.ascontiguousarray(wt),
                "xq": np.ascontiguousarray(xq),
            }
        )
    return in_maps


def kernel(inputs, W, _trace=False):
    nc = build_kernel()
    in_maps = _prep_inputs(np.asarray(inputs), np.asarray(W))
    res = run_bass_kernel_spmd(nc, in_maps, list(range(N_CORES)), trace=_trace)
    out = np.concatenate(
        [res.results[r]["o"].reshape(B, JL, P) for r in range(N_CORES)], axis=1
    )
    if _trace:
        kernel.last_exec_ns = res.exec_time_ns
        kernel.last_results = res
    return out.astype(np.float32)
